# revision 7
# baseline (speedup 1.0000x reference)
import os
import sys

for _p in ("/opt/trn_rl_repo", "/root/.axon_site/_ro/trn_rl_repo"):
    if os.path.isdir(_p) and _p not in sys.path:
        sys.path.insert(0, _p)

import numpy as np
import ml_dtypes

BF16 = ml_dtypes.bfloat16

C, H, W = 8, 2048, 2048
NSEG = 64
NCORES = 8
P = 128
ROWS_PER_CORE = H // NCORES          # 256
SH = ROWS_PER_CORE * W               # 524288 pixels per core
F = SH // P                          # 4096 free elements per partition
T = 512                              # max free-dim tile per pass
# small edge passes shrink the PE-idle pipeline fill/drain
PASS_SIZES = [640] * 6 + [256]
NPASS = len(PASS_SIZES)
TP = 8                               # pixels per matmul (lhsT = TP*16 = 128 cols)
SIGMA_AGG = 0.5

_CACHE = {}


def _build_bass():
    import concourse.bacc as bacc
    import concourse.mybir as mybir
    from concourse.tile import TileContext

    fp32 = mybir.dt.float32
    bf16 = mybir.dt.bfloat16
    Alu = mybir.AluOpType
    Act = mybir.ActivationFunctionType

    nc = bacc.Bacc("TRN2", target_bir_lowering=False, debug=False)

    fp8 = mybir.dt.float8e4
    # vb host layout: vb[p, f*16 + b*8 + c] = (pred if b==0 else pred*rmask)[c, p, f]
    vb_d = nc.dram_tensor("vbi", [P, F * 16], fp8, kind="ExternalInput")
    kl_d = nc.dram_tensor("klb", [P, F], bf16, kind="ExternalInput")

    seg_o = nc.dram_tensor("seg_out", [P, 63 * TP], fp32, kind="ExternalOutput")
    s1_o = nc.dram_tensor("s1_out", [P, NPASS], fp32, kind="ExternalOutput")

    T16 = T * 16

    with TileContext(nc) as tc:
        with (
            tc.tile_pool(name="res", bufs=1) as resp,
            tc.tile_pool(name="vb", bufs=2) as vbp,
            tc.tile_pool(name="oh", bufs=2) as ohp,
            tc.tile_pool(name="stage", bufs=2) as stagep,
            tc.tile_pool(name="scr", bufs=1) as scrp,
            tc.tile_pool(name="psum", bufs=1, space="PSUM") as psump,
        ):
            s1cols = resp.tile([P, NPASS], fp32)
            psum_t = psump.tile([P, 63 * TP], fp32, tag="main")

            # PE warmup: independent junk matmuls flip HAM to 2.4 GHz while
            # the first pass's DMA + one-hot build run
            junk = resp.tile([P, 512], bf16)
            nc.vector.memset(junk[:, :], 0.0)

            actpar = resp.tile([P, 7], fp32)
            for j, s in enumerate(range(61, 64)):
                nc.vector.memset(actpar[:, j : j + 1], float(-s))
            nc.vector.memset(actpar[:, 4:5], -16.0)
            nc.vector.memset(actpar[:, 5:6], 2.0)
            nc.vector.memset(actpar[:, 6:7], -1.0)
            psum_w = psump.tile([P, 504], fp32, tag="warm")
            for i in range(36):
                nc.tensor.matmul(
                    psum_w[:, :], lhsT=junk[:, 0:128], rhs=junk[:, 4:508],
                    start=True, stop=True,
                )

            off = 0
            for k, Tk in enumerate(PASS_SIZES):
                Tk16 = Tk * 16
                vb = vbp.tile([P, Tk16], fp8, tag="vb")
                vbv = vb.rearrange("p (t b c) -> p t b c", b=2, c=C)
                klb = stagep.tile([P, Tk], bf16, tag="kl")

                nc.sync.dma_start(klb[:, :], kl_d[:, off : off + Tk])
                nc.sync.dma_start(vb[:, :], vb_d[:, off * 16 : (off + Tk) * 16])

                # s1 partials: sum of Fp^2 over the Fp half, on the scalar engine
                scr8 = scrp.tile([P, Tk * C], bf16, tag="sq")
                nc.scalar.activation(
                    scr8.rearrange("p (t c) -> p t c", c=C),
                    vbv[:, :, 1, :],
                    Act.Square,
                    accum_out=s1cols[:, k : k + 1],
                )

                # one-hot planes for slots 1..63 (slot 0 is masked out of the
                # loss, so its plane is never needed). Slots 1..59 on DVE
                # (is_equal, 4x); slots 60..63 on ACT via exp(-16*(kl-s)^2),
                # exact to ~1e-7 for integer labels.
                oh = ohp.tile([P, NSEG * Tk], bf16, tag="oh")
                oh3 = oh.rearrange("p (s t) -> p s t", s=NSEG)
                for s in range(1, 61):
                    nc.vector.tensor_scalar(
                        oh3[:, s, :], klb[:, :], float(s), None, op0=Alu.is_equal
                    )
                for s in range(61, 64):
                    ysc = scrp.tile([P, Tk], bf16, tag=f"act{s % 2}")
                    nc.scalar.activation(
                        ysc[:, :], klb[:, :], Act.Square,
                        bias=actpar[:, s - 61 : s - 60],
                    )
                    ysc2 = scrp.tile([P, Tk], bf16, tag=f"actb{s % 2}")
                    nc.scalar.activation(
                        ysc2[:, :], ysc[:, :], Act.Exp, scale=actpar[:, 4:5]
                    )
                    nc.scalar.activation(
                        oh3[:, s, :], ysc2[:, :], Act.Relu,
                        scale=actpar[:, 5:6], bias=actpar[:, 6:7],
                    )

                # DoubleRow fp8 matmuls: contraction 256 = 128 partitions x
                # 2 k-tiles (pixel halves). The bf16 one-hot is consumed via its
                # fp8 byte view: odd bytes of bf16 {0.0, 1.0} are e4m3
                # {0.0, 1.875} exactly; host rescales by 1/1.875.
                ohf = oh.bitcast(fp8)
                ohv = ohf.rearrange(
                    "p (s kt g tr two) -> p g kt s tr two",
                    s=NSEG, kt=2, g=Tk // (2 * TP), tr=TP,
                )
                vbv8 = vb.rearrange("p (kt t m) -> p kt t m", kt=2, m=16)
                NG = Tk // (2 * TP)
                for g in range(NG):
                    nc.tensor.matmul(
                        psum_t[:, :],
                        lhsT=vbv8[:, :, g * TP : (g + 1) * TP, :],
                        rhs=ohv[:, g, :, 1:, :, 1],
                        perf_mode=mybir.MatmulPerfMode.DoubleRow,
                        start=(k == 0 and g == 0),
                        stop=(k == NPASS - 1 and g == NG - 1),
                    )
                off += Tk

            seg_sb = resp.tile([P, 63 * TP], fp32)
            nc.vector.tensor_copy(seg_sb[:, :], psum_t[:, :])
            nc.sync.dma_start(seg_o[:, :], seg_sb[:, :])
            nc.sync.dma_start(s1_o[:, :], s1cols[:, :])

    nc.compile()
    return nc


def _get_nc():
    if "nc" not in _CACHE:
        _CACHE["nc"] = _build_bass()
    return _CACHE["nc"]


def _to_bf16(x):
    """fp32 -> bf16 with round-to-nearest-even, fast numpy path."""
    x = np.ascontiguousarray(x, dtype=np.float32)
    u = x.view(np.uint32)
    r = (u >> 16) & 1
    ub = ((u + np.uint32(0x7FFF) + r) >> 16).astype(np.uint16)
    return ub.view(BF16).reshape(x.shape)


def _make_in_maps(pred, kl, rl):
    rmask01 = (rl > 0).astype(np.float32)
    FP8 = ml_dtypes.float8_e4m3
    predb = pred.astype(FP8)                        # [C, H, W]
    fpb = (pred * rmask01[None, :, :]).astype(FP8)  # [C, H, W]
    klb = kl.astype(BF16)
    in_maps = []
    for ci in range(NCORES):
        rows = slice(ci * ROWS_PER_CORE, (ci + 1) * ROWS_PER_CORE)
        predc = predb[:, rows, :].reshape(C, P, F).transpose(1, 2, 0)  # [P,F,C]
        fpc = fpb[:, rows, :].reshape(C, P, F).transpose(1, 2, 0)
        vb = np.ascontiguousarray(
            np.stack([predc, fpc], axis=2)                            # [P,F,2,C]
        ).reshape(P, F * 16)
        in_maps.append({
            "vbi": vb,
            "klb": np.ascontiguousarray(klb[rows, :]).reshape(P, F),
        })
    return in_maps


def _numpy_fallback(pred, rmask, kmask, kl, rl):
    klf = kl.reshape(-1)
    rlf = rl.reshape(-1)
    kcard = np.zeros(NSEG, np.float64)
    np.add.at(kcard, klf, kmask.reshape(-1).astype(np.float64))
    rcard = np.zeros(NSEG, np.float64)
    np.add.at(rcard, rlf, rmask.reshape(-1).astype(np.float64))
    predf = pred.reshape(C, -1).astype(np.float64)
    seg = np.zeros((C, NSEG), np.float64)
    for c in range(C):
        np.add.at(seg[c], klf, predf[c])
    g = np.where(np.arange(NSEG)[None, :] > 0, seg, 0.0) / (kcard + 1.0)[None, :]
    Fp = predf * rmask.reshape(-1)[None, :].astype(np.float64)
    diff = Fp - g[:, klf]
    D = max(np.sqrt(np.sum(diff * diff)) - SIGMA_AGG, 0.0)
    L = np.log(D * D + 1.0)
    pixsum = np.sum(1.0 / (rcard[rlf] + 1.0))
    num_region = max(rl.max(), 1)
    return np.float32(L * pixsum / num_region)


def kernel(**inputs):
    from concourse import bass_utils

    pred = np.asarray(inputs["pred_similarities"], dtype=np.float32)
    rmask = np.asarray(inputs["regions_mask"], dtype=np.float32)
    kmask = np.asarray(inputs["kernels_mask"], dtype=np.float32)
    kl = np.asarray(inputs["kernel_labels"], dtype=np.int32)
    rl = np.asarray(inputs["region_labels"], dtype=np.int32)

    if not np.array_equal(rmask, (rl > 0).astype(np.float32)) or not np.array_equal(
        kmask, (kl > 0).astype(np.float32)
    ):
        return _numpy_fallback(pred, rmask, kmask, kl, rl)

    nc = _get_nc()
    in_maps = _make_in_maps(pred, kl, rl)
    res = bass_utils.run_bass_kernel_spmd(nc, in_maps, core_ids=list(range(NCORES)))

    B = np.zeros((C, NSEG), np.float64)
    A = np.zeros((C, NSEG), np.float64)
    s1 = 0.0
    for r in res.results:
        # seg rows = (t_r*16 + b*8 + c), cols = ((s-1)*TP + t_r) for s in 1..63
        seg = r["seg_out"].astype(np.float64).reshape(TP, 2, C, 63, TP) / 1.875
        for tr in range(TP):
            B[:, 1:] += seg[tr, 0, :, :, tr]
            A[:, 1:] += seg[tr, 1, :, :, tr]
        s1 += r["s1_out"].astype(np.float64).sum()

    hist_k = np.bincount(kl.reshape(-1), minlength=NSEG).astype(np.float64)
    hist_r = np.bincount(rl.reshape(-1), minlength=NSEG).astype(np.float64)
    kcard = hist_k.copy()
    kcard[0] = 0.0

    mask_s = (np.arange(NSEG) > 0).astype(np.float64)
    g = mask_s[None, :] * B / (kcard + 1.0)[None, :]

    sumsq = s1 - 2.0 * np.sum(A * g) + np.sum(hist_k[None, :] * g * g)
    D = max(np.sqrt(max(sumsq, 0.0)) - SIGMA_AGG, 0.0)
    L = np.log(D * D + 1.0)
    rcard = hist_r.copy()
    rcard[0] = 0.0
    pixsum = np.sum(hist_r / (rcard + 1.0))
    num_region = max(float(rl.max()), 1.0)
    return np.float32(L * pixsum / num_region)
